# revision 61
# baseline (speedup 1.0000x reference)
"""Trainium2 Bass kernel for nn_Attention_85057532330254.

Self-attention block (conv1x1 QKV + BatchNorm, relative-position bias,
softmax, gelu, out-projection + BatchNorm), batch-sharded across 8 cores.

Transfer design (the axon tunnel dominates wall time):
 - x up in fp16, output down in int8 (scale 8/127, folded into BN2 affine).
 - All weights/vectors ship as ONE packed fp16 container per core holding a
   1/8 shard; the shard section is AllGathered on device.
 - The relative-position bias exp(bias/scale) is NOT shipped: it is
   block-Toeplitz, so each core ships one 32x2016 strip (its head) and the
   full per-head [1024,1024] B matrix is expanded on device by strided
   window DMAs (custom overlapping-window access patterns, 4 per head).

Device design (per core, 2 images = 2048 tokens):
 - x is XBAR-DMA-transposed to [channel, token] so BatchNorm stats are
   free-dim reductions and the BN affine is a per-partition scale/bias.
 - BN uses global batch stats -> two tiny AllReduces (qkv stats, z stats).
 - Softmax: exp(dots + bias) = exp(dots) * exp(bias); B multiplied in on
   DVE (fp16 2x rate).
 - Scores are built transposed (sT[j,i]) so attn@V needs no transposes;
   V_aug carries a ones-column producing softmax row-sums for free.
 - V's BN affine is folded into a deferred gelu pass over head-pair tiles
   (one ACT function-table switch); attention output is built transposed
   in head pairs so the output projection contracts K=128 per matmul.
 - BN2 stats via ones-column matmul reductions; second AllReduce; final
   affine (+int8 quantization) applied on DVE, result DMA'd out.
"""

import os

import numpy as np

import bass_rust
import concourse.bass as bass
import concourse.mybir as mybir
import concourse.tile as tile
from concourse import bacc
from concourse.bass import ts
from concourse.bass_utils import run_bass_kernel_spmd
F32 = mybir.dt.float32
F16 = mybir.dt.float16
BF16 = mybir.dt.bfloat16
I8 = mybir.dt.int8
OUT_Q = 127.0 / 8.0             # int8 output quantization scale

# packed fp16 weight container layout (per-core shard + replicated tail)
OFF_QKV = 0                     # [32, 1024]  wqkv rows 32c:32c+32
OFF_STRIP = 32768               # [32, 2016]  exp-bias Toeplitz strip, head c
OFF_WOUT = 97280                # [64, 256]   w_out rows 64c:64c+64
GATHER_N = 113664               # shard section, AllGathered on device
OFF_GB = 113664                 # [128, 16]   qkv BN gamma/beta (replicated)
OFF_VEC2 = 115712               # [1, 768]    b_out | go | bo   (replicated)
PK_N = 116480
AF = mybir.ActivationFunctionType
ALU = mybir.AluOpType

FMAP = 32
HEADS = 8
DK = 32
DV = 64
EPS = 1e-5
N_TOK = FMAP * FMAP            # 1024 tokens per image
DIM = 256
INNER_K = HEADS * DK           # 256
INNER_V = HEADS * DV           # 512
SCALE = DK ** -0.5
NCORES = 8
IMGS = 2                        # images per core
TOKS = IMGS * N_TOK             # 2048
NTOT = float(16 * N_TOK)        # global batch size for BN stats

_cache = {}


def _enable_jax_compile_cache():
    # run_bass_via_pjrt builds a fresh jit closure per call, so the in-memory
    # executable cache misses every time and the NEFF pipeline reruns (~0.4s).
    # The persistent cache is keyed by HLO hash, identical across calls.
    try:
        import jax
        jax.config.update("jax_compilation_cache_dir", "/tmp/jax_comp_cache")
        jax.config.update("jax_persistent_cache_min_compile_time_secs", 0)
        jax.config.update("jax_persistent_cache_min_entry_size_bytes", 0)
    except Exception:
        pass


def _build():
    from contextlib import ExitStack

    ndev = 1 if os.environ.get("KTIME") else NCORES
    nc = bacc.Bacc(
        "TRN2", target_bir_lowering=False, debug=False, num_devices=ndev
    )
    x_d = nc.dram_tensor("x", [TOKS, DIM], F16, kind="ExternalInput").ap()
    # single packed fp16 container: per-core 1/8 weight shards (wqkv rows,
    # exp-bias Toeplitz strip for head c, w_out rows — AllGathered on device)
    # plus the replicated BN vectors. Strip block 31+s (s in [-31,31]) is the
    # [32,32] tile T_{|s|}[yj,yi] = exp(pos_emb[|s|*32+|yj-yi|, h]/scale);
    # block-row xj of the [1024,1024] bias matrix B[h] is the contiguous
    # 1024-column strip window starting at block 31-xj.
    wpk_d = nc.dram_tensor("wpk", [1, PK_N], F16, kind="ExternalInput").ap()
    # output quantized to int8 (scale 8/127); halves readback + donated zeros
    out_d = nc.dram_tensor("out", [TOKS, DIM], I8, kind="ExternalOutput").ap()

    with tile.TileContext(nc) as tc, ExitStack() as es:
        _kernel_body(tc, es, x_d, wpk_d, out_d)
    nc.compile()
    return nc


def _kernel_body(tc, es, x_d, wpk_d, out_d):
    nc = tc.nc
    RG = [list(range(NCORES))]

    # AllGather the 1/8 weight shards to the full container in local DRAM
    gdram = es.enter_context(tc.tile_pool(name="gdram", bufs=1, space="DRAM"))
    wg = gdram.tile([NCORES, GATHER_N], F16)
    if os.environ.get("KTIME"):
        for k in range(NCORES):
            nc.sync.dma_start(wg[k:k + 1, :], wpk_d[:, 0:GATHER_N])
    else:
        # collectives may not read IO tensors: stage shard in internal DRAM
        stg = gdram.tile([1, GATHER_N], F16)
        nc.sync.dma_start(stg[:], wpk_d[:, 0:GATHER_N])
        nc.gpsimd.collective_compute(
            "AllGather", ALU.bypass, replica_groups=RG,
            ins=[stg[:].opt()], outs=[wg[:].opt()],
        )
    wqv = wg[:, OFF_QKV:OFF_QKV + 32768].rearrange("h (p c) -> h p c", p=32)
    wsv = wg[:, OFF_STRIP:OFF_STRIP + 64512].rearrange("h (p c) -> h p c", p=32)
    wov = wg[:, OFF_WOUT:OFF_WOUT + 16384].rearrange("h (p c) -> h p c", p=64)

    const = es.enter_context(tc.tile_pool(name="const", bufs=1))
    gb16 = const.tile([128, 16], F16)
    nc.sync.dma_start(
        gb16[:], wpk_d[:, OFF_GB:OFF_GB + 2048].rearrange("o (p c) -> o p c", p=128)
    )
    gb_sb = const.tile([128, 16], F32)
    nc.vector.tensor_copy(out=gb_sb[:], in_=gb16[:])
    v16 = const.tile([1, 3 * DIM], F16)
    nc.sync.dma_start(v16[:], wpk_d[:, OFF_VEC2:OFF_VEC2 + 768])
    vec2_sb = const.tile([1, 3 * DIM], F32)
    nc.vector.tensor_copy(out=vec2_sb[:], in_=v16[:])
    onescol = const.tile([128, 1], F32)
    nc.gpsimd.memset(onescol[:], 1.0)

    # persistent activations; g tiles hold head-PAIRS on the partition axis
    # (head 2d in partitions 0-63, head 2d+1 in 64-127) so the output
    # projection contracts K=128 per matmul.
    big = es.enter_context(tc.tile_pool(name="big", bufs=1))
    QKb = [big.tile([128, TOKS], BF16, tag=f"qkb{i}", name=f"qkb{i}") for i in range(4)]
    V_aug = big.tile([128, 16, HEADS, DV + 2], F16, name="vaug")
    gXp = [big.tile([128, TOKS], F16, tag=f"gx{i}", name=f"gx{i}") for i in range(4)]
    gTp = [big.tile([128, TOKS], F16, tag=f"gt{i}", name=f"gt{i}") for i in range(4)]
    z_sb = big.tile([128, 16 * DIM], F32, name="z_sb")
    stats_sb = const.tile([128, 16], F32)
    stats_all = const.tile([128, 16], F32)
    scale_t = const.tile([128, 8], F32)
    bias_t = const.tile([128, 8], F32)

    # ---------------- phase A/B: load x transposed, project, stats --------
    # XBAR DMA transpose: x [2048 tok, 128 ch-chunk] -> XT [128 ch, 2048 tok]
    xtp = tc.tile_pool(name="xtp", bufs=1)
    xtpool = xtp.__enter__()
    XT = [xtpool.tile([128, TOKS], F16, tag=f"xt{i}", name=f"xt{i}") for i in range(2)]
    for fc in range(2):
        nc.sync.dma_start_transpose(XT[fc][:], x_d[:, ts(fc, 128)])

    wq_sb = [const.tile([128, 1024], F16, tag=f"wq{i}", name=f"wq{i}") for i in range(2)]
    for kc in range(2):
        for j in range(4):
            nc.sync.dma_start(
                wq_sb[kc][32 * j:32 * j + 32, :], wqv[4 * kc + j]
            )
    # w_out in head-pairs: wop[d] = wout rows [128d : 128d+128]
    wop = [const.tile([128, DIM], F16, tag=f"wo{i}", name=f"wo{i}") for i in range(4)]
    for dc in range(4):
        nc.sync.dma_start(wop[dc][0:64, :], wov[2 * dc])
        nc.sync.dma_start(wop[dc][64:128, :], wov[2 * dc + 1])

    # projections chunk-by-chunk: c8 = q0 q1 k0 k1 v0 v1 v2 v3
    with (
        tc.tile_pool(name="qkraw", bufs=1) as qkraw_pool,
        tc.tile_pool(name="scratch", bufs=1) as scratch_pool,
    ):
        qkraw = []
        with tc.tile_pool(name="projps", bufs=2, space="PSUM") as projps:
          for c8 in range(8):
            ps = projps.tile([128, TOKS], F32, tag="proj")
            for ns in range(4):
                for kc in range(2):
                    nc.tensor.matmul(
                        ps[:, ts(ns, 512)],
                        lhsT=wq_sb[kc][:, ts(c8, 128)],
                        rhs=XT[kc][:, ts(ns, 512)],
                        start=(kc == 0),
                        stop=(kc == 1),
                    )
            scr = scratch_pool.tile([128, TOKS], BF16, tag="sq")
            nc.scalar.activation(
                out=scr[:], in_=ps[:], func=AF.Square,
                accum_out=stats_sb[:, 8 + c8:9 + c8],
            )
            nc.vector.tensor_reduce(
                out=stats_sb[:, c8:c8 + 1], in_=ps[:],
                axis=mybir.AxisListType.X, op=ALU.add,
            )
            if c8 < 4:
                raw = qkraw_pool.tile([128, TOKS], F32, tag=f"qk{c8}")
                nc.vector.tensor_copy(out=raw[:], in_=ps[:])
                qkraw.append(raw)

        # V natural (for attn@V lhsT): tiles [128tok, heads, 2+64];
        # col 65 = ones column producing softmax row-sums
        nc.gpsimd.memset(V_aug[:, :, :, 65:66], 1.0)
        with tc.tile_pool(name="vps", bufs=2, space="PSUM") as vps:
            for t in range(16):
                ps = vps.tile([128, INNER_V], F32)
                for kc in range(2):
                    nc.tensor.matmul(
                        ps[:],
                        lhsT=XT[kc][:, ts(t, 128)],
                        rhs=wq_sb[kc][:, 512:1024],
                        start=(kc == 0),
                        stop=(kc == 1),
                    )
                nc.vector.tensor_copy(
                    out=V_aug[:, t, :, 1:65],
                    in_=ps.rearrange("p (h d) -> p h d", h=HEADS),
                )

        # ---- AllReduce 1: 2048 floats of (sum, sumsq) ----
        with tc.tile_pool(name="dram1", bufs=1, space="DRAM") as dram1:
            cin = dram1.tile([128, 16], F32)
            cout = dram1.tile([128, 16], F32)
            nc.sync.dma_start(cin[:], stats_sb[:])
            if os.environ.get("KTIME"):
                nc.sync.dma_start(cout[:], cin[:])
            else:
                nc.gpsimd.collective_compute(
                    "AllReduce", ALU.add, replica_groups=RG,
                    ins=[cin[:].opt()], outs=[cout[:].opt()],
                )
            nc.sync.dma_start(stats_all[:], cout[:])

        # ---- finalize BN1 affine: scale_t/bias_t [128, 8] ----
        mean = const.tile([128, 8], F32)
        ex2 = const.tile([128, 8], F32)
        veps = const.tile([128, 8], F32)
        sq0 = const.tile([128, 8], F32)
        tmp = const.tile([128, 8], F32)
        rstd = const.tile([128, 8], F32)
        nc.vector.tensor_scalar_mul(mean[:], stats_all[:, 0:8], 1.0 / NTOT)
        nc.vector.tensor_scalar_mul(ex2[:], stats_all[:, 8:16], 1.0 / NTOT)
        # veps = ex2 - mean^2 + eps
        nc.vector.scalar_tensor_tensor(
            out=tmp[:], in0=mean[:], scalar=-1.0, in1=mean[:],
            op0=ALU.mult, op1=ALU.mult,
        )
        nc.vector.tensor_add(veps[:], ex2[:], tmp[:])
        nc.vector.tensor_scalar_add(veps[:], veps[:], EPS)
        nc.scalar.sqrt(sq0[:], veps[:])
        nc.vector.reciprocal(rstd[:], sq0[:])
        # scale = gamma * rstd ; bias = beta - mean * scale
        nc.vector.tensor_mul(scale_t[:], gb_sb[:, 0:8], rstd[:])
        nc.vector.scalar_tensor_tensor(
            out=tmp[:], in0=mean[:], scalar=-1.0, in1=scale_t[:],
            op0=ALU.mult, op1=ALU.mult,
        )
        nc.vector.tensor_add(bias_t[:], gb_sb[:, 8:16], tmp[:])
        # fold attention 1/sqrt(dk) into q
        nc.vector.tensor_scalar_mul(scale_t[:, 0:2], scale_t[:, 0:2], SCALE)
        nc.vector.tensor_scalar_mul(bias_t[:, 0:2], bias_t[:, 0:2], SCALE)

        # normalize Q/K -> bf16 (per-partition affine on ACT)
        for c8 in range(4):
            nc.scalar.activation(
                out=QKb[c8][:], in_=qkraw[c8][:], func=AF.Identity,
                bias=bias_t[:, c8:c8 + 1], scale=scale_t[:, c8:c8 + 1],
            )

        # repack per-head V scale/bias into pair layout: col d = head pair,
        # partitions 0-63 = head 2d, 64-127 = head 2d+1
        svp = const.tile([128, 4], F32)
        bvp = const.tile([128, 4], F32)
        for h in range(HEADS):
            lo = 64 * (h % 2)
            c = 4 + h // 2
            d = h // 2
            nc.sync.dma_start(svp[lo:lo + 64, d:d + 1], scale_t[lo:lo + 64, c:c + 1])
            nc.sync.dma_start(bvp[lo:lo + 64, d:d + 1], bias_t[lo:lo + 64, c:c + 1])

    xtp.__exit__(None, None, None)

    # ---------------- phase C: attention ----------------------------------
    with (
        tc.tile_pool(name="bpool", bufs=2) as bpool,
        tc.tile_pool(name="stpool", bufs=11) as stpool,
        tc.tile_pool(name="expool", bufs=3) as expool,
        tc.tile_pool(name="aps", bufs=2, space="PSUM") as aps,
        tc.tile_pool(name="small", bufs=4) as small,
    ):
        for h in range(HEADS):
            qk_t = h // 4
            hp = h % 4
            # expand bias strips -> full per-head B [128(j), jc, i]: the
            # partition group a = xj%4 of chunk jc holds strip window
            # starting at block 31-xj (xj = 4*jc + a).
            # 4 custom-AP DMAs: src strip[yj, (31-4jc-a)*32+c] over (yj,jc,c)
            # (overlapping jc windows, stride -128) -> dst B[32a+yj, jc, c]
            B_sb = bpool.tile([128, 8 * N_TOK], F16, tag="B")
            VP = bass_rust.VecI64Pair
            for a in range(4):
                src = wsv[h].copy()
                src.ap = VP([(2016, 32), (-128, 8), (1, N_TOK)])
                src.offset = wsv[h].offset + 992 - 32 * a
                dst = B_sb[:].copy()
                dst.ap = VP([(8 * N_TOK, 32), (N_TOK, 8), (1, N_TOK)])
                dst.offset = B_sb[:].offset + 32 * a * 8 * N_TOK
                # SWDGE (Pool queue): keeps these off the in-order SP DMA
                # queue, which stalls behind the stats-collective chain
                nc.gpsimd.dma_start(dst, src)
            sT = []
            for jc in range(8):
                st = stpool.tile([128, 2 * N_TOK], F16, tag="sT")
                kpos = 32 * hp
                tp = (96, 0) if hp == 3 else None
                for img in range(IMGS):
                    dots = aps.tile([128, N_TOK], F32, tag="dots")
                    for ih in range(2):
                        nc.tensor.matmul(
                            dots[:, ts(ih, 512)],
                            lhsT=QKb[2 + qk_t][kpos:kpos + 32,
                                               img * N_TOK + jc * 128:
                                               img * N_TOK + jc * 128 + 128],
                            rhs=QKb[qk_t][kpos:kpos + 32,
                                          img * N_TOK + ih * 512:
                                          img * N_TOK + ih * 512 + 512],
                            start=True, stop=True,
                            tile_position=tp,
                        )
                    ex = expool.tile([128, N_TOK], F16, tag="exp")
                    nc.scalar.activation(out=ex[:], in_=dots[:], func=AF.Exp)
                    nc.vector.tensor_mul(
                        st[:, ts(img, N_TOK)], ex[:], B_sb[:, ts(jc, N_TOK)],
                    )
                sT.append(st)
            half = 64 * (h % 2)
            d = h // 2
            for img in range(IMGS):
                # attn @ V_aug: rows 0..63 = dv, row 64 = rowsum (ones col)
                outp = aps.tile([128, N_TOK], F32, tag="outT", name="outp")
                for ih in range(2):
                    for jc in range(8):
                        nc.tensor.matmul(
                            outp[0:65, ts(ih, 512)],
                            lhsT=V_aug[:, img * 8 + jc, h, 1:66],
                            rhs=sT[jc][:, img * N_TOK + ih * 512:
                                       img * N_TOK + ih * 512 + 512],
                            start=(jc == 0), stop=(jc == 7),
                        )
                rowi = small.tile([1, N_TOK], F32, tag="rowi")
                nc.vector.reciprocal(rowi[:], outp[64:65, :])
                bc = small.tile([64, N_TOK], F32, tag="bc")
                nc.gpsimd.partition_broadcast(bc[:], rowi[0:1, :])
                nc.vector.tensor_mul(
                    gXp[d][half:half + 64, ts(img, N_TOK)], outp[0:64, :], bc[:]
                )

        # deferred gelu over head-pair tiles: one ActFuncSet switch total,
        # per-partition scale/bias = folded V BatchNorm affine
        for d in range(4):
            nc.scalar.activation(
                out=gTp[d][:], in_=gXp[d][:], func=AF.Gelu_apprx_tanh,
                bias=bvp[:, d:d + 1], scale=svp[:, d:d + 1],
            )

    # ---------------- phase D: out-projection + BN2 ------------------------
    with (
        tc.tile_pool(name="zps", bufs=2, space="PSUM") as zps,
        tc.tile_pool(name="sps", bufs=1, space="PSUM") as sps,
        tc.tile_pool(name="zmisc", bufs=2) as zmisc,
        tc.tile_pool(name="dram2", bufs=1, space="DRAM") as dram2,
        tc.tile_pool(name="fin", bufs=1) as fin,
    ):
        sums_ps = sps.tile([1, 2 * DIM], F32)
        for t in range(16):
            ps = zps.tile([128, DIM], F32, tag="z")
            for dc in range(4):
                nc.tensor.matmul(
                    ps[:],
                    lhsT=gTp[dc][:, ts(t, 128)],
                    rhs=wop[dc][:],
                    start=(dc == 0), stop=(dc == 3),
                )
            nc.vector.tensor_copy(out=z_sb[:, ts(t, DIM)], in_=ps[:])
            z2 = zmisc.tile([128, DIM], F32, tag="z2")
            nc.gpsimd.tensor_mul(z2[:], z_sb[:, ts(t, DIM)], z_sb[:, ts(t, DIM)])
            nc.tensor.matmul(
                sums_ps[0:1, 0:DIM], lhsT=onescol[:], rhs=z_sb[:, ts(t, DIM)],
                start=(t == 0), stop=(t == 15), skip_group_check=True,
            )
            nc.tensor.matmul(
                sums_ps[0:1, DIM:2 * DIM], lhsT=onescol[:], rhs=z2[:],
                start=(t == 0), stop=(t == 15), skip_group_check=True,
            )
        st2 = fin.tile([1, 2 * DIM], F32)
        nc.vector.tensor_copy(out=st2[:], in_=sums_ps[:])
        cin = dram2.tile([1, 2 * DIM], F32)
        cout = dram2.tile([1, 2 * DIM], F32)
        nc.sync.dma_start(cin[:], st2[:])
        if os.environ.get("KTIME"):
            nc.sync.dma_start(cout[:], cin[:])
        else:
            nc.gpsimd.collective_compute(
                "AllReduce", ALU.add, replica_groups=RG,
                ins=[cin[:].opt()], outs=[cout[:].opt()],
            )
        st2a = fin.tile([1, 2 * DIM], F32)
        nc.sync.dma_start(st2a[:], cout[:])

        # finalize BN2 on [1, 256] rows.  z_true = z_raw + b_out
        mean = fin.tile([1, DIM], F32)
        ex2 = fin.tile([1, DIM], F32)
        veps = fin.tile([1, DIM], F32)
        sq0 = fin.tile([1, DIM], F32)
        tmp = fin.tile([1, DIM], F32)
        s2 = fin.tile([1, DIM], F32)
        b2f = fin.tile([1, DIM], F32)
        b_out_row = vec2_sb[0:1, 0:DIM]
        go_row = vec2_sb[0:1, DIM:2 * DIM]
        bo_row = vec2_sb[0:1, 2 * DIM:3 * DIM]
        nc.vector.tensor_scalar_mul(mean[:], st2a[0:1, 0:DIM], 1.0 / NTOT)
        nc.vector.tensor_scalar_mul(ex2[:], st2a[0:1, DIM:2 * DIM], 1.0 / NTOT)
        # ex2_true = ex2 + 2*mean*b_out + b_out^2 ; m_true = mean + b_out
        nc.vector.scalar_tensor_tensor(
            out=tmp[:], in0=mean[:], scalar=2.0, in1=b_out_row,
            op0=ALU.mult, op1=ALU.mult,
        )
        nc.vector.tensor_add(ex2[:], ex2[:], tmp[:])
        nc.vector.tensor_mul(tmp[:], b_out_row, b_out_row)
        nc.vector.tensor_add(ex2[:], ex2[:], tmp[:])
        m_true = fin.tile([1, DIM], F32)
        nc.vector.tensor_add(m_true[:], mean[:], b_out_row)
        nc.vector.scalar_tensor_tensor(
            out=tmp[:], in0=m_true[:], scalar=-1.0, in1=m_true[:],
            op0=ALU.mult, op1=ALU.mult,
        )
        nc.vector.tensor_add(veps[:], ex2[:], tmp[:])
        nc.vector.tensor_scalar_add(veps[:], veps[:], EPS)
        nc.scalar.sqrt(sq0[:], veps[:])
        nc.vector.reciprocal(tmp[:], sq0[:])        # rstd2
        nc.vector.tensor_mul(s2[:], go_row, tmp[:])
        # bias2_final = bo - mean_raw * s2
        nc.vector.scalar_tensor_tensor(
            out=tmp[:], in0=mean[:], scalar=-1.0, in1=s2[:],
            op0=ALU.mult, op1=ALU.mult,
        )
        nc.vector.tensor_add(b2f[:], bo_row, tmp[:])
        # fold int8 output quantization into the BN2 affine
        nc.vector.tensor_scalar_mul(s2[:], s2[:], OUT_Q)
        nc.vector.tensor_scalar_mul(b2f[:], b2f[:], OUT_Q)
        # widen scale/bias rows 16x (step-0 DMA), broadcast across partitions
        s2w = fin.tile([1, 16 * DIM], F32)
        b2w = fin.tile([1, 16 * DIM], F32)
        nc.sync.dma_start(
            s2w.rearrange("o (r c) -> o r c", r=16),
            s2.rearrange("o (u c) -> o u c", u=1).broadcast_to((1, 16, DIM)),
        )
        nc.sync.dma_start(
            b2w.rearrange("o (r c) -> o r c", r=16),
            b2f.rearrange("o (u c) -> o u c", u=1).broadcast_to((1, 16, DIM)),
        )
        bcs2 = fin.tile([128, 16 * DIM], F32)
        bcb2 = fin.tile([128, 16 * DIM], F32)
        nc.gpsimd.partition_broadcast(bcs2[:], s2w[:])
        nc.gpsimd.partition_broadcast(bcb2[:], b2w[:])
        zt = fin.tile([128, 16 * DIM], F32)
        nc.vector.tensor_mul(zt[:], z_sb[:], bcs2[:])
        zo = fin.tile([128, 16 * DIM], I8)
        nc.vector.tensor_add(zo[:], zt[:], bcb2[:])
        nc.sync.dma_start(
            out_d.rearrange("(t p) c -> p t c", p=128), zo.rearrange("p (t c) -> p t c", t=16)
        )


def kernel(**inputs):
    f = np.float32
    x = np.asarray(inputs["x"], f)
    wq, wk, wv = (np.asarray(inputs[k], f) for k in ("wq", "wk", "wv"))
    pos_emb = np.asarray(inputs["pos_emb"], f)
    w_out = np.asarray(inputs["w_out"], f)

    # Toeplitz strips of exp(bias/scale): [yj, t=31+s, yi, h] with
    # value exp(pos_emb[|s|*32 + |yj-yi|, h] / SCALE); core c carries head c.
    E = np.exp(pos_emb / SCALE)                      # [1024, h]
    dy = np.abs(np.arange(32)[:, None] - np.arange(32)[None, :])  # [yj, yi]
    dxs = np.abs(np.arange(63) - 31)                 # [t]
    idx = dxs[None, :, None] * 32 + dy[:, None, :]   # [yj, t, yi]
    strips = E[idx].astype(np.float16)               # [yj, t, yi, h]

    wqkv = np.concatenate([wq, wk, wv], axis=1).astype(np.float16)  # [256, 1024]
    wout16 = w_out.astype(np.float16)
    # gb: col 0-7 gammas, 8-15 betas, chunk order q0 q1 k0 k1 v0..v3
    gcat = np.concatenate(
        [np.asarray(inputs["gq"], f), np.asarray(inputs["gk"], f),
         np.asarray(inputs["gv"], f)]
    ).reshape(8, 128).T
    bcat = np.concatenate(
        [np.asarray(inputs["bq"], f), np.asarray(inputs["bk"], f),
         np.asarray(inputs["bv"], f)]
    ).reshape(8, 128).T
    gb = np.concatenate([gcat, bcat], axis=1).astype(np.float16)  # [128, 16]
    vec2 = np.concatenate(
        [np.asarray(inputs["b_out"], f), np.asarray(inputs["go"], f),
         np.asarray(inputs["bo"], f)]
    ).astype(np.float16)                              # [768]

    if "nc" not in _cache:
        _enable_jax_compile_cache()
        _cache["nc"] = _build()
    nc = _cache["nc"]

    xs = x.reshape(16, N_TOK, DIM).astype(np.float16)
    in_maps = []
    for c in range(NCORES):
        pk = np.empty((1, PK_N), np.float16)
        pk[0, OFF_QKV:OFF_QKV + 32768] = wqkv[32 * c:32 * c + 32].ravel()
        pk[0, OFF_STRIP:OFF_STRIP + 64512] = strips[:, :, :, c].ravel()
        pk[0, OFF_WOUT:OFF_WOUT + 16384] = wout16[64 * c:64 * c + 64].ravel()
        pk[0, OFF_GB:OFF_GB + 2048] = gb.ravel()
        pk[0, OFF_VEC2:OFF_VEC2 + 768] = vec2
        in_maps.append({
            "x": np.ascontiguousarray(xs[2 * c:2 * c + 2].reshape(TOKS, DIM)),
            "wpk": pk,
        })

    res = run_bass_kernel_spmd(
        nc, in_maps, core_ids=list(range(NCORES)),
        trace=bool(int(os.environ.get("KTRACE", "0"))),
    )
    _cache["res"] = res
    out = np.concatenate([r["out"] for r in res.results], axis=0)
    return (out.astype(np.float32) / OUT_Q).reshape(16, FMAP, FMAP, DIM)


if __name__ == "__main__":
    if os.environ.get("BUILD_ONLY"):
        _build()
        print("BUILD OK")



# revision 66
# speedup vs baseline: 1.0485x; 1.0485x over previous
"""Trainium2 Bass kernel for nn_Attention_85057532330254.

Self-attention block (conv1x1 QKV + BatchNorm, relative-position bias,
softmax, gelu, out-projection + BatchNorm), batch-sharded across 8 cores.

Transfer design (the axon tunnel dominates wall time):
 - x up in fp16, output down in int8 (scale 8/127, folded into BN2 affine).
 - All weights/vectors ship as ONE packed fp16 container per core holding a
   1/8 shard; the shard section is AllGathered on device.
 - The relative-position bias exp(bias/scale) is NOT shipped: it is
   block-Toeplitz, so each core ships one 32x2016 strip (its head) and the
   full per-head [1024,1024] B matrix is expanded on device by strided
   window DMAs (custom overlapping-window access patterns, 4 per head).

Device design (per core, 2 images = 2048 tokens):
 - x is XBAR-DMA-transposed to [channel, token] so BatchNorm stats are
   free-dim reductions and the BN affine is a per-partition scale/bias.
 - BN uses global batch stats -> two tiny AllReduces (qkv stats, z stats).
 - Softmax: exp(dots + bias) = exp(dots) * exp(bias); B multiplied in on
   DVE (fp16 2x rate).
 - Scores are built transposed (sT[j,i]) so attn@V needs no transposes;
   V_aug carries a ones-column producing softmax row-sums for free.
 - V's BN affine is folded into a deferred gelu pass over head-pair tiles
   (one ACT function-table switch); attention output is built transposed
   in head pairs so the output projection contracts K=128 per matmul.
 - BN2 stats via ones-column matmul reductions; second AllReduce; final
   affine (+int8 quantization) applied on DVE, result DMA'd out.
"""

import os

import numpy as np

import bass_rust
import concourse.bass as bass
import concourse.mybir as mybir
import concourse.tile as tile
from concourse import bacc
from concourse.bass import ts
from concourse.bass_utils import run_bass_kernel_spmd
F32 = mybir.dt.float32
F16 = mybir.dt.float16
BF16 = mybir.dt.bfloat16
I8 = mybir.dt.int8
OUT_Q = 127.0 / 8.0             # int8 output quantization scale

# packed fp16 weight container layout (per-core shard + replicated tail)
OFF_QKV = 0                     # [32, 1024]  wqkv rows 32c:32c+32
OFF_STRIP = 32768               # [32, 2016]  exp-bias Toeplitz strip, head c
OFF_WOUT = 97280                # [64, 256]   w_out rows 64c:64c+64
GATHER_N = 113664               # shard section, AllGathered on device
OFF_GB = 113664                 # [128, 16]   qkv BN gamma/beta (replicated)
OFF_VEC2 = 115712               # [1, 768]    b_out | go | bo   (replicated)
PK_N = 116480
AF = mybir.ActivationFunctionType
ALU = mybir.AluOpType

FMAP = 32
HEADS = 8
DK = 32
DV = 64
EPS = 1e-5
N_TOK = FMAP * FMAP            # 1024 tokens per image
DIM = 256
INNER_K = HEADS * DK           # 256
INNER_V = HEADS * DV           # 512
SCALE = DK ** -0.5
NCORES = 8
IMGS = 2                        # images per core
TOKS = IMGS * N_TOK             # 2048
NTOT = float(16 * N_TOK)        # global batch size for BN stats

_cache = {}


def _enable_jax_compile_cache():
    # run_bass_via_pjrt builds a fresh jit closure per call, so the in-memory
    # executable cache misses every time and the NEFF pipeline reruns (~0.4s).
    # The persistent cache is keyed by HLO hash, identical across calls.
    try:
        import jax
        jax.config.update("jax_compilation_cache_dir", "/tmp/jax_comp_cache")
        jax.config.update("jax_persistent_cache_min_compile_time_secs", 0)
        jax.config.update("jax_persistent_cache_min_entry_size_bytes", 0)
    except Exception:
        pass


def _build():
    from contextlib import ExitStack

    ndev = 1 if os.environ.get("KTIME") else NCORES
    nc = bacc.Bacc(
        "TRN2", target_bir_lowering=False, debug=False, num_devices=ndev
    )
    x_d = nc.dram_tensor("x", [TOKS, DIM], F16, kind="ExternalInput").ap()
    # single packed fp16 container: per-core 1/8 weight shards (wqkv rows,
    # exp-bias Toeplitz strip for head c, w_out rows — AllGathered on device)
    # plus the replicated BN vectors. Strip block 31+s (s in [-31,31]) is the
    # [32,32] tile T_{|s|}[yj,yi] = exp(pos_emb[|s|*32+|yj-yi|, h]/scale);
    # block-row xj of the [1024,1024] bias matrix B[h] is the contiguous
    # 1024-column strip window starting at block 31-xj.
    wpk_d = nc.dram_tensor("wpk", [1, PK_N], F16, kind="ExternalInput").ap()
    # output quantized to int8 (scale 8/127); halves readback + donated zeros
    out_d = nc.dram_tensor("out", [TOKS, DIM], I8, kind="ExternalOutput").ap()

    with tile.TileContext(nc) as tc, ExitStack() as es:
        _kernel_body(tc, es, x_d, wpk_d, out_d)
    nc.compile()
    return nc


def _kernel_body(tc, es, x_d, wpk_d, out_d):
    nc = tc.nc
    RG = [list(range(NCORES))]

    # AllGather the 1/8 weight shards to the full container in local DRAM
    gdram = es.enter_context(tc.tile_pool(name="gdram", bufs=1, space="DRAM"))
    wg = gdram.tile([NCORES, GATHER_N], F16)
    if os.environ.get("KTIME"):
        # single-core stand-in for the AllGather: one broadcast-AP DMA
        nc.sync.dma_start(
            wg[:], wpk_d[0:1, 0:GATHER_N].broadcast_to((NCORES, GATHER_N))
        )
    else:
        # collectives may not read IO tensors: stage shard in internal DRAM
        stg = gdram.tile([1, GATHER_N], F16)
        nc.sync.dma_start(stg[:], wpk_d[:, 0:GATHER_N])
        nc.gpsimd.collective_compute(
            "AllGather", ALU.bypass, replica_groups=RG,
            ins=[stg[:].opt()], outs=[wg[:].opt()],
        )
    wqv = wg[:, OFF_QKV:OFF_QKV + 32768].rearrange("h (p c) -> h p c", p=32)
    wsv = wg[:, OFF_STRIP:OFF_STRIP + 64512].rearrange("h (p c) -> h p c", p=32)
    wov = wg[:, OFF_WOUT:OFF_WOUT + 16384].rearrange("h (p c) -> h p c", p=64)

    const = es.enter_context(tc.tile_pool(name="const", bufs=1))
    gb16 = const.tile([128, 16], F16)
    nc.sync.dma_start(
        gb16[:], wpk_d[:, OFF_GB:OFF_GB + 2048].rearrange("o (p c) -> o p c", p=128)
    )
    gb_sb = const.tile([128, 16], F32)
    nc.vector.tensor_copy(out=gb_sb[:], in_=gb16[:])
    v16 = const.tile([1, 3 * DIM], F16)
    nc.sync.dma_start(v16[:], wpk_d[:, OFF_VEC2:OFF_VEC2 + 768])
    vec2_sb = const.tile([1, 3 * DIM], F32)
    nc.vector.tensor_copy(out=vec2_sb[:], in_=v16[:])
    onescol = const.tile([128, 1], F32)
    nc.gpsimd.memset(onescol[:], 1.0)

    # persistent activations; g tiles hold head-PAIRS on the partition axis
    # (head 2d in partitions 0-63, head 2d+1 in 64-127) so the output
    # projection contracts K=128 per matmul.
    big = es.enter_context(tc.tile_pool(name="big", bufs=1))
    QKb = [big.tile([128, TOKS], BF16, tag=f"qkb{i}", name=f"qkb{i}") for i in range(4)]
    V_aug = big.tile([128, 16, HEADS, DV + 2], F16, name="vaug")
    gXp = [big.tile([128, TOKS], F16, tag=f"gx{i}", name=f"gx{i}") for i in range(4)]
    gTp = [big.tile([128, TOKS], F16, tag=f"gt{i}", name=f"gt{i}") for i in range(4)]
    z_sb = big.tile([128, 16 * DIM], F32, name="z_sb")
    stats_sb = const.tile([128, 16], F32)
    stats_all = const.tile([128, 16], F32)
    scale_t = const.tile([128, 8], F32)
    bias_t = const.tile([128, 8], F32)

    # ---------------- phase A/B: load x transposed, project, stats --------
    # XBAR DMA transpose: x [2048 tok, 128 ch-chunk] -> XT [128 ch, 2048 tok]
    xtp = tc.tile_pool(name="xtp", bufs=1)
    xtpool = xtp.__enter__()
    XT = [xtpool.tile([128, TOKS], F16, tag=f"xt{i}", name=f"xt{i}") for i in range(2)]
    for fc in range(2):
        nc.sync.dma_start_transpose(XT[fc][:], x_d[:, ts(fc, 128)])

    wq_sb = [const.tile([128, 1024], F16, tag=f"wq{i}", name=f"wq{i}") for i in range(2)]
    for kc in range(2):
        for j in range(4):
            nc.sync.dma_start(
                wq_sb[kc][32 * j:32 * j + 32, :], wqv[4 * kc + j]
            )
    # w_out in head-pairs: wop[d] = wout rows [128d : 128d+128]
    wop = [const.tile([128, DIM], F16, tag=f"wo{i}", name=f"wo{i}") for i in range(4)]
    for dc in range(4):
        nc.sync.dma_start(wop[dc][0:64, :], wov[2 * dc])
        nc.sync.dma_start(wop[dc][64:128, :], wov[2 * dc + 1])

    # projections chunk-by-chunk: c8 = q0 q1 k0 k1 v0 v1 v2 v3
    with (
        tc.tile_pool(name="qkraw", bufs=1) as qkraw_pool,
        tc.tile_pool(name="scratch", bufs=1) as scratch_pool,
    ):
        qkraw = []
        with tc.tile_pool(name="projps", bufs=2, space="PSUM") as projps:
          for c8 in range(8):
            ps = projps.tile([128, TOKS], F32, tag="proj")
            for ns in range(4):
                for kc in range(2):
                    nc.tensor.matmul(
                        ps[:, ts(ns, 512)],
                        lhsT=wq_sb[kc][:, ts(c8, 128)],
                        rhs=XT[kc][:, ts(ns, 512)],
                        start=(kc == 0),
                        stop=(kc == 1),
                    )
            scr = scratch_pool.tile([128, TOKS], BF16, tag="sq")
            nc.scalar.activation(
                out=scr[:], in_=ps[:], func=AF.Square,
                accum_out=stats_sb[:, 8 + c8:9 + c8],
            )
            # Identity+accum gives the raw copy AND the per-partition sum in
            # one ACT pass (no separate DVE reduce/copy)
            if c8 < 4:
                raw = qkraw_pool.tile([128, TOKS], F32, tag=f"qk{c8}")
                qkraw.append(raw)
                sum_dst = raw[:]
            else:
                dump = scratch_pool.tile([128, TOKS], BF16, tag="dump", name="dump")
                sum_dst = dump[:]
            nc.scalar.activation(
                out=sum_dst, in_=ps[:], func=AF.Identity,
                accum_out=stats_sb[:, c8:c8 + 1],
            )

        # V natural (for attn@V lhsT): tiles [128tok, heads, 2+64];
        # col 65 = ones column producing softmax row-sums
        nc.gpsimd.memset(V_aug[:, :, :, 65:66], 1.0)
        with tc.tile_pool(name="vps", bufs=2, space="PSUM") as vps:
            for t in range(16):
                ps = vps.tile([128, INNER_V], F32)
                for kc in range(2):
                    nc.tensor.matmul(
                        ps[:],
                        lhsT=XT[kc][:, ts(t, 128)],
                        rhs=wq_sb[kc][:, 512:1024],
                        start=(kc == 0),
                        stop=(kc == 1),
                    )
                nc.vector.tensor_copy(
                    out=V_aug[:, t, :, 1:65],
                    in_=ps.rearrange("p (h d) -> p h d", h=HEADS),
                )

        # ---- AllReduce 1: 2048 floats of (sum, sumsq) ----
        with tc.tile_pool(name="dram1", bufs=1, space="DRAM") as dram1:
            cin = dram1.tile([128, 16], F32)
            cout = dram1.tile([128, 16], F32)
            nc.sync.dma_start(cin[:], stats_sb[:])
            if os.environ.get("KTIME"):
                nc.sync.dma_start(cout[:], cin[:])
            else:
                nc.gpsimd.collective_compute(
                    "AllReduce", ALU.add, replica_groups=RG,
                    ins=[cin[:].opt()], outs=[cout[:].opt()],
                )
            nc.sync.dma_start(stats_all[:], cout[:])

        # ---- finalize BN1 affine: scale_t/bias_t [128, 8] ----
        mean = const.tile([128, 8], F32)
        ex2 = const.tile([128, 8], F32)
        veps = const.tile([128, 8], F32)
        sq0 = const.tile([128, 8], F32)
        tmp = const.tile([128, 8], F32)
        rstd = const.tile([128, 8], F32)
        nc.vector.tensor_scalar_mul(mean[:], stats_all[:, 0:8], 1.0 / NTOT)
        nc.vector.tensor_scalar_mul(ex2[:], stats_all[:, 8:16], 1.0 / NTOT)
        # veps = ex2 - mean^2 + eps
        nc.vector.scalar_tensor_tensor(
            out=tmp[:], in0=mean[:], scalar=-1.0, in1=mean[:],
            op0=ALU.mult, op1=ALU.mult,
        )
        nc.vector.tensor_add(veps[:], ex2[:], tmp[:])
        nc.vector.tensor_scalar_add(veps[:], veps[:], EPS)
        nc.scalar.sqrt(sq0[:], veps[:])
        nc.vector.reciprocal(rstd[:], sq0[:])
        # scale = gamma * rstd ; bias = beta - mean * scale
        nc.vector.tensor_mul(scale_t[:], gb_sb[:, 0:8], rstd[:])
        nc.vector.scalar_tensor_tensor(
            out=tmp[:], in0=mean[:], scalar=-1.0, in1=scale_t[:],
            op0=ALU.mult, op1=ALU.mult,
        )
        nc.vector.tensor_add(bias_t[:], gb_sb[:, 8:16], tmp[:])
        # fold attention 1/sqrt(dk) into q
        nc.vector.tensor_scalar_mul(scale_t[:, 0:2], scale_t[:, 0:2], SCALE)
        nc.vector.tensor_scalar_mul(bias_t[:, 0:2], bias_t[:, 0:2], SCALE)

        # normalize Q/K -> bf16 (per-partition affine on ACT)
        for c8 in range(4):
            nc.scalar.activation(
                out=QKb[c8][:], in_=qkraw[c8][:], func=AF.Identity,
                bias=bias_t[:, c8:c8 + 1], scale=scale_t[:, c8:c8 + 1],
            )

        # repack per-head V scale/bias into pair layout: col d = head pair,
        # partitions 0-63 = head 2d, 64-127 = head 2d+1
        svp = const.tile([128, 4], F32)
        bvp = const.tile([128, 4], F32)
        for h in range(HEADS):
            lo = 64 * (h % 2)
            c = 4 + h // 2
            d = h // 2
            nc.sync.dma_start(svp[lo:lo + 64, d:d + 1], scale_t[lo:lo + 64, c:c + 1])
            nc.sync.dma_start(bvp[lo:lo + 64, d:d + 1], bias_t[lo:lo + 64, c:c + 1])

    xtp.__exit__(None, None, None)

    # ---------------- phase C: attention ----------------------------------
    with (
        tc.tile_pool(name="bpool", bufs=2) as bpool,
        tc.tile_pool(name="stpool", bufs=11) as stpool,
        tc.tile_pool(name="expool", bufs=3) as expool,
        tc.tile_pool(name="aps", bufs=2, space="PSUM") as aps,
        tc.tile_pool(name="small", bufs=4) as small,
    ):
        for h in range(HEADS):
            qk_t = h // 4
            hp = h % 4
            # expand bias strips -> full per-head B [128(j), jc, i]: the
            # partition group a = xj%4 of chunk jc holds strip window
            # starting at block 31-xj (xj = 4*jc + a).
            # 4 custom-AP DMAs: src strip[yj, (31-4jc-a)*32+c] over (yj,jc,c)
            # (overlapping jc windows, stride -128) -> dst B[32a+yj, jc, c]
            B_sb = bpool.tile([128, 8 * N_TOK], F16, tag="B")
            VP = bass_rust.VecI64Pair
            for a in range(4):
                src = wsv[h].copy()
                src.ap = VP([(2016, 32), (-128, 8), (1, N_TOK)])
                src.offset = wsv[h].offset + 992 - 32 * a
                dst = B_sb[:].copy()
                dst.ap = VP([(8 * N_TOK, 32), (N_TOK, 8), (1, N_TOK)])
                dst.offset = B_sb[:].offset + 32 * a * 8 * N_TOK
                # SWDGE (Pool queue): keeps these off the in-order SP DMA
                # queue, which stalls behind the stats-collective chain
                nc.gpsimd.dma_start(dst, src)
            sT = []
            for jc in range(8):
                st = stpool.tile([128, 2 * N_TOK], F16, tag="sT")
                kpos = 32 * hp
                tp = (96, 0) if hp == 3 else None
                for img in range(IMGS):
                    dots = aps.tile([128, N_TOK], F32, tag="dots")
                    for ih in range(2):
                        nc.tensor.matmul(
                            dots[:, ts(ih, 512)],
                            lhsT=QKb[2 + qk_t][kpos:kpos + 32,
                                               img * N_TOK + jc * 128:
                                               img * N_TOK + jc * 128 + 128],
                            rhs=QKb[qk_t][kpos:kpos + 32,
                                          img * N_TOK + ih * 512:
                                          img * N_TOK + ih * 512 + 512],
                            start=True, stop=True,
                            tile_position=tp,
                        )
                    ex = expool.tile([128, N_TOK], F16, tag="exp")
                    nc.scalar.activation(out=ex[:], in_=dots[:], func=AF.Exp)
                    nc.vector.tensor_mul(
                        st[:, ts(img, N_TOK)], ex[:], B_sb[:, ts(jc, N_TOK)],
                    )
                sT.append(st)
            half = 64 * (h % 2)
            d = h // 2
            for img in range(IMGS):
                # attn @ V_aug: rows 0..63 = dv, row 64 = rowsum (ones col)
                outp = aps.tile([128, N_TOK], F32, tag="outT", name="outp")
                for ih in range(2):
                    for jc in range(8):
                        nc.tensor.matmul(
                            outp[0:65, ts(ih, 512)],
                            lhsT=V_aug[:, img * 8 + jc, h, 1:66],
                            rhs=sT[jc][:, img * N_TOK + ih * 512:
                                       img * N_TOK + ih * 512 + 512],
                            start=(jc == 0), stop=(jc == 7),
                        )
                rowi = small.tile([1, N_TOK], F32, tag="rowi")
                nc.vector.reciprocal(rowi[:], outp[64:65, :])
                bc = small.tile([64, N_TOK], F32, tag="bc")
                nc.gpsimd.partition_broadcast(bc[:], rowi[0:1, :])
                nc.vector.tensor_mul(
                    gXp[d][half:half + 64, ts(img, N_TOK)], outp[0:64, :], bc[:]
                )

        # deferred gelu over head-pair tiles: one ActFuncSet switch total,
        # per-partition scale/bias = folded V BatchNorm affine
        for d in range(4):
            nc.scalar.activation(
                out=gTp[d][:], in_=gXp[d][:], func=AF.Gelu_apprx_tanh,
                bias=bvp[:, d:d + 1], scale=svp[:, d:d + 1],
            )

    # ---------------- phase D: out-projection + BN2 ------------------------
    with (
        tc.tile_pool(name="zps", bufs=2, space="PSUM") as zps,
        tc.tile_pool(name="sps", bufs=1, space="PSUM") as sps,
        tc.tile_pool(name="zmisc", bufs=2) as zmisc,
        tc.tile_pool(name="dram2", bufs=1, space="DRAM") as dram2,
        tc.tile_pool(name="fin", bufs=1) as fin,
    ):
        sums_ps = sps.tile([1, 2 * DIM], F32)
        for t in range(16):
            ps = zps.tile([128, DIM], F32, tag="z")
            for dc in range(4):
                nc.tensor.matmul(
                    ps[:],
                    lhsT=gTp[dc][:, ts(t, 128)],
                    rhs=wop[dc][:],
                    start=(dc == 0), stop=(dc == 3),
                )
            nc.vector.tensor_copy(out=z_sb[:, ts(t, DIM)], in_=ps[:])
            z2 = zmisc.tile([128, DIM], F32, tag="z2")
            nc.gpsimd.tensor_mul(z2[:], z_sb[:, ts(t, DIM)], z_sb[:, ts(t, DIM)])
            nc.tensor.matmul(
                sums_ps[0:1, 0:DIM], lhsT=onescol[:], rhs=z_sb[:, ts(t, DIM)],
                start=(t == 0), stop=(t == 15), skip_group_check=True,
            )
            nc.tensor.matmul(
                sums_ps[0:1, DIM:2 * DIM], lhsT=onescol[:], rhs=z2[:],
                start=(t == 0), stop=(t == 15), skip_group_check=True,
            )
        st2 = fin.tile([1, 2 * DIM], F32)
        nc.vector.tensor_copy(out=st2[:], in_=sums_ps[:])
        cin = dram2.tile([1, 2 * DIM], F32)
        cout = dram2.tile([1, 2 * DIM], F32)
        nc.sync.dma_start(cin[:], st2[:])
        if os.environ.get("KTIME"):
            nc.sync.dma_start(cout[:], cin[:])
        else:
            nc.gpsimd.collective_compute(
                "AllReduce", ALU.add, replica_groups=RG,
                ins=[cin[:].opt()], outs=[cout[:].opt()],
            )
        st2a = fin.tile([1, 2 * DIM], F32)
        nc.sync.dma_start(st2a[:], cout[:])

        # finalize BN2 on [1, 256] rows.  z_true = z_raw + b_out
        mean = fin.tile([1, DIM], F32)
        ex2 = fin.tile([1, DIM], F32)
        veps = fin.tile([1, DIM], F32)
        sq0 = fin.tile([1, DIM], F32)
        tmp = fin.tile([1, DIM], F32)
        s2 = fin.tile([1, DIM], F32)
        b2f = fin.tile([1, DIM], F32)
        b_out_row = vec2_sb[0:1, 0:DIM]
        go_row = vec2_sb[0:1, DIM:2 * DIM]
        bo_row = vec2_sb[0:1, 2 * DIM:3 * DIM]
        nc.vector.tensor_scalar_mul(mean[:], st2a[0:1, 0:DIM], 1.0 / NTOT)
        nc.vector.tensor_scalar_mul(ex2[:], st2a[0:1, DIM:2 * DIM], 1.0 / NTOT)
        # ex2_true = ex2 + 2*mean*b_out + b_out^2 ; m_true = mean + b_out
        nc.vector.scalar_tensor_tensor(
            out=tmp[:], in0=mean[:], scalar=2.0, in1=b_out_row,
            op0=ALU.mult, op1=ALU.mult,
        )
        nc.vector.tensor_add(ex2[:], ex2[:], tmp[:])
        nc.vector.tensor_mul(tmp[:], b_out_row, b_out_row)
        nc.vector.tensor_add(ex2[:], ex2[:], tmp[:])
        m_true = fin.tile([1, DIM], F32)
        nc.vector.tensor_add(m_true[:], mean[:], b_out_row)
        nc.vector.scalar_tensor_tensor(
            out=tmp[:], in0=m_true[:], scalar=-1.0, in1=m_true[:],
            op0=ALU.mult, op1=ALU.mult,
        )
        nc.vector.tensor_add(veps[:], ex2[:], tmp[:])
        nc.vector.tensor_scalar_add(veps[:], veps[:], EPS)
        nc.scalar.sqrt(sq0[:], veps[:])
        nc.vector.reciprocal(tmp[:], sq0[:])        # rstd2
        nc.vector.tensor_mul(s2[:], go_row, tmp[:])
        # bias2_final = bo - mean_raw * s2
        nc.vector.scalar_tensor_tensor(
            out=tmp[:], in0=mean[:], scalar=-1.0, in1=s2[:],
            op0=ALU.mult, op1=ALU.mult,
        )
        nc.vector.tensor_add(b2f[:], bo_row, tmp[:])
        # fold int8 output quantization into the BN2 affine
        nc.vector.tensor_scalar_mul(s2[:], s2[:], OUT_Q)
        nc.vector.tensor_scalar_mul(b2f[:], b2f[:], OUT_Q)
        # widen scale/bias rows 16x (step-0 DMA), broadcast across partitions
        s2w = fin.tile([1, 16 * DIM], F32)
        b2w = fin.tile([1, 16 * DIM], F32)
        nc.sync.dma_start(
            s2w.rearrange("o (r c) -> o r c", r=16),
            s2.rearrange("o (u c) -> o u c", u=1).broadcast_to((1, 16, DIM)),
        )
        nc.sync.dma_start(
            b2w.rearrange("o (r c) -> o r c", r=16),
            b2f.rearrange("o (u c) -> o u c", u=1).broadcast_to((1, 16, DIM)),
        )
        bcs2 = fin.tile([128, 16 * DIM], F32)
        bcb2 = fin.tile([128, 16 * DIM], F32)
        nc.gpsimd.partition_broadcast(bcs2[:], s2w[:])
        nc.gpsimd.partition_broadcast(bcb2[:], b2w[:])
        # apply affine + int8 quantization in 4 chunks so the output DMAs
        # overlap the remaining DVE work instead of trailing one big op
        zt = fin.tile([128, 16 * DIM], F32)
        zo = fin.tile([128, 16 * DIM], I8)
        odv = out_d.rearrange("(t p) c -> p t c", p=128)
        zov = zo.rearrange("p (t c) -> p t c", t=16)
        CH = 4 * DIM
        for q in range(4):
            nc.vector.tensor_mul(
                zt[:, ts(q, CH)], z_sb[:, ts(q, CH)], bcs2[:, ts(q, CH)]
            )
            nc.vector.tensor_add(
                zo[:, ts(q, CH)], zt[:, ts(q, CH)], bcb2[:, ts(q, CH)]
            )
            nc.sync.dma_start(odv[:, 4 * q:4 * q + 4, :], zov[:, 4 * q:4 * q + 4, :])


def kernel(**inputs):
    f = np.float32
    x = np.asarray(inputs["x"], f)
    wq, wk, wv = (np.asarray(inputs[k], f) for k in ("wq", "wk", "wv"))
    pos_emb = np.asarray(inputs["pos_emb"], f)
    w_out = np.asarray(inputs["w_out"], f)

    # Toeplitz strips of exp(bias/scale): [yj, t=31+s, yi, h] with
    # value exp(pos_emb[|s|*32 + |yj-yi|, h] / SCALE); core c carries head c.
    E = np.exp(pos_emb / SCALE)                      # [1024, h]
    dy = np.abs(np.arange(32)[:, None] - np.arange(32)[None, :])  # [yj, yi]
    dxs = np.abs(np.arange(63) - 31)                 # [t]
    idx = dxs[None, :, None] * 32 + dy[:, None, :]   # [yj, t, yi]
    strips = E[idx].astype(np.float16)               # [yj, t, yi, h]

    wqkv = np.concatenate([wq, wk, wv], axis=1).astype(np.float16)  # [256, 1024]
    wout16 = w_out.astype(np.float16)
    # gb: col 0-7 gammas, 8-15 betas, chunk order q0 q1 k0 k1 v0..v3
    gcat = np.concatenate(
        [np.asarray(inputs["gq"], f), np.asarray(inputs["gk"], f),
         np.asarray(inputs["gv"], f)]
    ).reshape(8, 128).T
    bcat = np.concatenate(
        [np.asarray(inputs["bq"], f), np.asarray(inputs["bk"], f),
         np.asarray(inputs["bv"], f)]
    ).reshape(8, 128).T
    gb = np.concatenate([gcat, bcat], axis=1).astype(np.float16)  # [128, 16]
    vec2 = np.concatenate(
        [np.asarray(inputs["b_out"], f), np.asarray(inputs["go"], f),
         np.asarray(inputs["bo"], f)]
    ).astype(np.float16)                              # [768]

    if "nc" not in _cache:
        _enable_jax_compile_cache()
        _cache["nc"] = _build()
    nc = _cache["nc"]

    xs = x.reshape(16, N_TOK, DIM).astype(np.float16)
    in_maps = []
    for c in range(NCORES):
        pk = np.empty((1, PK_N), np.float16)
        pk[0, OFF_QKV:OFF_QKV + 32768] = wqkv[32 * c:32 * c + 32].ravel()
        pk[0, OFF_STRIP:OFF_STRIP + 64512] = strips[:, :, :, c].ravel()
        pk[0, OFF_WOUT:OFF_WOUT + 16384] = wout16[64 * c:64 * c + 64].ravel()
        pk[0, OFF_GB:OFF_GB + 2048] = gb.ravel()
        pk[0, OFF_VEC2:OFF_VEC2 + 768] = vec2
        in_maps.append({
            "x": np.ascontiguousarray(xs[2 * c:2 * c + 2].reshape(TOKS, DIM)),
            "wpk": pk,
        })

    res = run_bass_kernel_spmd(
        nc, in_maps, core_ids=list(range(NCORES)),
        trace=bool(int(os.environ.get("KTRACE", "0"))),
    )
    _cache["res"] = res
    out = np.concatenate([r["out"] for r in res.results], axis=0)
    return (out.astype(np.float32) / OUT_Q).reshape(16, FMAP, FMAP, DIM)


if __name__ == "__main__":
    if os.environ.get("BUILD_ONLY"):
        _build()
        print("BUILD OK")



# revision 67
# speedup vs baseline: 1.0787x; 1.0287x over previous
"""Trainium2 Bass kernel for nn_Attention_85057532330254.

Self-attention block (conv1x1 QKV + BatchNorm, relative-position bias,
softmax, gelu, out-projection + BatchNorm), batch-sharded across 8 cores.

Transfer design (the axon tunnel dominates wall time):
 - x up in fp16, output down in int8 (scale 8/127, folded into BN2 affine).
 - All weights/vectors ship as ONE packed fp16 container per core holding a
   1/8 shard; the shard section is AllGathered on device.
 - The relative-position bias exp(bias/scale) is NOT shipped: it is
   block-Toeplitz, so each core ships one 32x2016 strip (its head) and the
   full per-head [1024,1024] B matrix is expanded on device by strided
   window DMAs (custom overlapping-window access patterns, 4 per head).

Device design (per core, 2 images = 2048 tokens):
 - x is XBAR-DMA-transposed to [channel, token] so BatchNorm stats are
   free-dim reductions and the BN affine is a per-partition scale/bias.
 - BN uses global batch stats -> two tiny AllReduces (qkv stats, z stats).
 - Softmax: exp(dots + bias) = exp(dots) * exp(bias); B multiplied in on
   DVE (fp16 2x rate).
 - Scores are built transposed (sT[j,i]) so attn@V needs no transposes;
   V_aug carries a ones-column producing softmax row-sums for free.
 - V's BN affine is folded into a deferred gelu pass over head-pair tiles
   (one ACT function-table switch); attention output is built transposed
   in head pairs so the output projection contracts K=128 per matmul.
 - BN2 stats via ones-column matmul reductions; second AllReduce; final
   affine (+int8 quantization) applied on DVE, result DMA'd out.
"""

import os

import numpy as np

import bass_rust
import concourse.bass as bass
import concourse.mybir as mybir
import concourse.tile as tile
from concourse import bacc
from concourse.bass import ts
from concourse.bass_utils import run_bass_kernel_spmd
F32 = mybir.dt.float32
F16 = mybir.dt.float16
BF16 = mybir.dt.bfloat16
I8 = mybir.dt.int8
OUT_Q = 127.0 / 8.0             # int8 output quantization scale

# packed fp16 weight container layout (per-core shard + replicated tail)
OFF_QKV = 0                     # [32, 1024]  wqkv rows 32c:32c+32
OFF_STRIP = 32768               # [32, 2016]  exp-bias Toeplitz strip, head c
OFF_WOUT = 97280                # [64, 256]   w_out rows 64c:64c+64
GATHER_N = 113664               # shard section, AllGathered on device
OFF_GB = 113664                 # [128, 16]   qkv BN gamma/beta (replicated)
OFF_VEC2 = 115712               # [1, 768]    b_out | go | bo   (replicated)
PK_N = 116480
AF = mybir.ActivationFunctionType
ALU = mybir.AluOpType

FMAP = 32
HEADS = 8
DK = 32
DV = 64
EPS = 1e-5
N_TOK = FMAP * FMAP            # 1024 tokens per image
DIM = 256
INNER_K = HEADS * DK           # 256
INNER_V = HEADS * DV           # 512
SCALE = DK ** -0.5
NCORES = 8
IMGS = 2                        # images per core
TOKS = IMGS * N_TOK             # 2048
NTOT = float(16 * N_TOK)        # global batch size for BN stats

_cache = {}


def _enable_jax_compile_cache():
    # run_bass_via_pjrt builds a fresh jit closure per call, so the in-memory
    # executable cache misses every time and the NEFF pipeline reruns (~0.4s).
    # The persistent cache is keyed by HLO hash, identical across calls.
    try:
        import jax
        jax.config.update("jax_compilation_cache_dir", "/tmp/jax_comp_cache")
        jax.config.update("jax_persistent_cache_min_compile_time_secs", 0)
        jax.config.update("jax_persistent_cache_min_entry_size_bytes", 0)
    except Exception:
        pass


def _build():
    from contextlib import ExitStack

    ndev = 1 if os.environ.get("KTIME") else NCORES
    nc = bacc.Bacc(
        "TRN2", target_bir_lowering=False, debug=False, num_devices=ndev
    )
    x_d = nc.dram_tensor("x", [TOKS, DIM], F16, kind="ExternalInput").ap()
    # single packed fp16 container: per-core 1/8 weight shards (wqkv rows,
    # exp-bias Toeplitz strip for head c, w_out rows — AllGathered on device)
    # plus the replicated BN vectors. Strip block 31+s (s in [-31,31]) is the
    # [32,32] tile T_{|s|}[yj,yi] = exp(pos_emb[|s|*32+|yj-yi|, h]/scale);
    # block-row xj of the [1024,1024] bias matrix B[h] is the contiguous
    # 1024-column strip window starting at block 31-xj.
    wpk_d = nc.dram_tensor("wpk", [1, PK_N], F16, kind="ExternalInput").ap()
    # output quantized to int8 (scale 8/127); halves readback + donated zeros
    out_d = nc.dram_tensor("out", [TOKS, DIM], I8, kind="ExternalOutput").ap()

    with tile.TileContext(nc) as tc, ExitStack() as es:
        _kernel_body(tc, es, x_d, wpk_d, out_d)
    nc.compile()
    return nc


def _kernel_body(tc, es, x_d, wpk_d, out_d):
    nc = tc.nc
    RG = [list(range(NCORES))]

    # AllGather the 1/8 weight shards to the full container in local DRAM
    gdram = es.enter_context(tc.tile_pool(name="gdram", bufs=1, space="DRAM"))
    wg = gdram.tile([NCORES, GATHER_N], F16)
    if os.environ.get("KTIME"):
        # single-core stand-in for the AllGather: one broadcast-AP DMA
        nc.sync.dma_start(
            wg[:], wpk_d[0:1, 0:GATHER_N].broadcast_to((NCORES, GATHER_N))
        )
    else:
        # collectives may not read IO tensors: stage shard in internal DRAM
        stg = gdram.tile([1, GATHER_N], F16)
        nc.sync.dma_start(stg[:], wpk_d[:, 0:GATHER_N])
        nc.gpsimd.collective_compute(
            "AllGather", ALU.bypass, replica_groups=RG,
            ins=[stg[:].opt()], outs=[wg[:].opt()],
        )
    wqv = wg[:, OFF_QKV:OFF_QKV + 32768].rearrange("h (p c) -> h p c", p=32)
    wsv = wg[:, OFF_STRIP:OFF_STRIP + 64512].rearrange("h (p c) -> h p c", p=32)
    wov = wg[:, OFF_WOUT:OFF_WOUT + 16384].rearrange("h (p c) -> h p c", p=64)

    const = es.enter_context(tc.tile_pool(name="const", bufs=1))
    gb16 = const.tile([128, 16], F16)
    nc.sync.dma_start(
        gb16[:], wpk_d[:, OFF_GB:OFF_GB + 2048].rearrange("o (p c) -> o p c", p=128)
    )
    gb_sb = const.tile([128, 16], F32)
    nc.vector.tensor_copy(out=gb_sb[:], in_=gb16[:])
    v16 = const.tile([1, 3 * DIM], F16)
    nc.sync.dma_start(v16[:], wpk_d[:, OFF_VEC2:OFF_VEC2 + 768])
    vec2_sb = const.tile([1, 3 * DIM], F32)
    nc.vector.tensor_copy(out=vec2_sb[:], in_=v16[:])
    onescol = const.tile([128, 1], F32)
    nc.gpsimd.memset(onescol[:], 1.0)

    # persistent activations; g tiles hold head-PAIRS on the partition axis
    # (head 2d in partitions 0-63, head 2d+1 in 64-127) so the output
    # projection contracts K=128 per matmul.
    big = es.enter_context(tc.tile_pool(name="big", bufs=1))
    QKb = [big.tile([128, TOKS], BF16, tag=f"qkb{i}", name=f"qkb{i}") for i in range(4)]
    V_aug = big.tile([128, 16, HEADS, DV + 2], F16, name="vaug")
    gXp = [big.tile([128, TOKS], F16, tag=f"gx{i}", name=f"gx{i}") for i in range(4)]
    gTp = [big.tile([128, TOKS], F16, tag=f"gt{i}", name=f"gt{i}") for i in range(4)]
    z_sb = big.tile([128, 16 * DIM], F32, name="z_sb")
    stats_sb = const.tile([128, 16], F32)
    stats_all = const.tile([128, 16], F32)
    scale_t = const.tile([128, 8], F32)
    bias_t = const.tile([128, 8], F32)

    # ---------------- phase A/B: load x transposed, project, stats --------
    # XBAR DMA transpose: x [2048 tok, 128 ch-chunk] -> XT [128 ch, 2048 tok]
    xtp = tc.tile_pool(name="xtp", bufs=1)
    xtpool = xtp.__enter__()
    XT = [xtpool.tile([128, TOKS], F16, tag=f"xt{i}", name=f"xt{i}") for i in range(2)]
    for fc in range(2):
        nc.sync.dma_start_transpose(XT[fc][:], x_d[:, ts(fc, 128)])

    wq_sb = [const.tile([128, 1024], F16, tag=f"wq{i}", name=f"wq{i}") for i in range(2)]
    for kc in range(2):
        for j in range(4):
            nc.sync.dma_start(
                wq_sb[kc][32 * j:32 * j + 32, :], wqv[4 * kc + j]
            )
    # w_out in head-pairs: wop[d] = wout rows [128d : 128d+128]
    wop = [const.tile([128, DIM], F16, tag=f"wo{i}", name=f"wo{i}") for i in range(4)]
    for dc in range(4):
        nc.sync.dma_start(wop[dc][0:64, :], wov[2 * dc])
        nc.sync.dma_start(wop[dc][64:128, :], wov[2 * dc + 1])

    # projections chunk-by-chunk: c8 = q0 q1 k0 k1 v0 v1 v2 v3
    with (
        tc.tile_pool(name="qkraw", bufs=1) as qkraw_pool,
        tc.tile_pool(name="scratch", bufs=1) as scratch_pool,
    ):
        qkraw = []
        with tc.tile_pool(name="projps", bufs=2, space="PSUM") as projps:
          for c8 in range(8):
            ps = projps.tile([128, TOKS], F32, tag="proj")
            for ns in range(4):
                for kc in range(2):
                    nc.tensor.matmul(
                        ps[:, ts(ns, 512)],
                        lhsT=wq_sb[kc][:, ts(c8, 128)],
                        rhs=XT[kc][:, ts(ns, 512)],
                        start=(kc == 0),
                        stop=(kc == 1),
                    )
            scr = scratch_pool.tile([128, TOKS], BF16, tag="sq")
            nc.scalar.activation(
                out=scr[:], in_=ps[:], func=AF.Square,
                accum_out=stats_sb[:, 8 + c8:9 + c8],
            )
            # Identity+accum gives the raw copy AND the per-partition sum in
            # one ACT pass (no separate DVE reduce/copy)
            if c8 < 4:
                raw = qkraw_pool.tile([128, TOKS], F32, tag=f"qk{c8}")
                qkraw.append(raw)
                sum_dst = raw[:]
            else:
                dump = scratch_pool.tile([128, TOKS], BF16, tag="dump", name="dump")
                sum_dst = dump[:]
            nc.scalar.activation(
                out=sum_dst, in_=ps[:], func=AF.Identity,
                accum_out=stats_sb[:, c8:c8 + 1],
            )

        # V natural (for attn@V lhsT): tiles [128tok, heads, 2+64];
        # col 65 = ones column producing softmax row-sums
        nc.gpsimd.memset(V_aug[:, :, :, 65:66], 1.0)
        with tc.tile_pool(name="vps", bufs=2, space="PSUM") as vps:
            for t in range(16):
                ps = vps.tile([128, INNER_V], F32)
                for kc in range(2):
                    nc.tensor.matmul(
                        ps[:],
                        lhsT=XT[kc][:, ts(t, 128)],
                        rhs=wq_sb[kc][:, 512:1024],
                        start=(kc == 0),
                        stop=(kc == 1),
                    )
                nc.vector.tensor_copy(
                    out=V_aug[:, t, :, 1:65],
                    in_=ps.rearrange("p (h d) -> p h d", h=HEADS),
                )

        # ---- AllReduce 1: 2048 floats of (sum, sumsq) ----
        with tc.tile_pool(name="dram1", bufs=1, space="DRAM") as dram1:
            cin = dram1.tile([128, 16], F32)
            cout = dram1.tile([128, 16], F32)
            nc.sync.dma_start(cin[:], stats_sb[:])
            if os.environ.get("KTIME"):
                nc.sync.dma_start(cout[:], cin[:])
            else:
                nc.gpsimd.collective_compute(
                    "AllReduce", ALU.add, replica_groups=RG,
                    ins=[cin[:].opt()], outs=[cout[:].opt()],
                )
            nc.sync.dma_start(stats_all[:], cout[:])

        # ---- finalize BN1 affine: scale_t/bias_t [128, 8] ----
        mean = const.tile([128, 8], F32)
        ex2 = const.tile([128, 8], F32)
        veps = const.tile([128, 8], F32)
        sq0 = const.tile([128, 8], F32)
        tmp = const.tile([128, 8], F32)
        rstd = const.tile([128, 8], F32)
        nc.vector.tensor_scalar_mul(mean[:], stats_all[:, 0:8], 1.0 / NTOT)
        nc.vector.tensor_scalar_mul(ex2[:], stats_all[:, 8:16], 1.0 / NTOT)
        # veps = ex2 - mean^2 + eps
        nc.vector.scalar_tensor_tensor(
            out=tmp[:], in0=mean[:], scalar=-1.0, in1=mean[:],
            op0=ALU.mult, op1=ALU.mult,
        )
        nc.vector.tensor_add(veps[:], ex2[:], tmp[:])
        nc.vector.tensor_scalar_add(veps[:], veps[:], EPS)
        nc.scalar.sqrt(sq0[:], veps[:])
        nc.vector.reciprocal(rstd[:], sq0[:])
        # scale = gamma * rstd ; bias = beta - mean * scale
        nc.vector.tensor_mul(scale_t[:], gb_sb[:, 0:8], rstd[:])
        nc.vector.scalar_tensor_tensor(
            out=tmp[:], in0=mean[:], scalar=-1.0, in1=scale_t[:],
            op0=ALU.mult, op1=ALU.mult,
        )
        nc.vector.tensor_add(bias_t[:], gb_sb[:, 8:16], tmp[:])
        # fold attention 1/sqrt(dk) into q
        nc.vector.tensor_scalar_mul(scale_t[:, 0:2], scale_t[:, 0:2], SCALE)
        nc.vector.tensor_scalar_mul(bias_t[:, 0:2], bias_t[:, 0:2], SCALE)

        # normalize Q/K -> bf16 (per-partition affine on ACT)
        for c8 in range(4):
            nc.scalar.activation(
                out=QKb[c8][:], in_=qkraw[c8][:], func=AF.Identity,
                bias=bias_t[:, c8:c8 + 1], scale=scale_t[:, c8:c8 + 1],
            )

        # repack per-head V scale/bias into pair layout: col d = head pair,
        # partitions 0-63 = head 2d, 64-127 = head 2d+1
        svp = const.tile([128, 4], F32)
        bvp = const.tile([128, 4], F32)
        for h in range(HEADS):
            lo = 64 * (h % 2)
            c = 4 + h // 2
            d = h // 2
            nc.sync.dma_start(svp[lo:lo + 64, d:d + 1], scale_t[lo:lo + 64, c:c + 1])
            nc.sync.dma_start(bvp[lo:lo + 64, d:d + 1], bias_t[lo:lo + 64, c:c + 1])

    xtp.__exit__(None, None, None)

    # ---------------- phase C: attention ----------------------------------
    with (
        tc.tile_pool(name="bpool", bufs=2) as bpool,
        tc.tile_pool(name="stpool", bufs=11) as stpool,
        tc.tile_pool(name="expool", bufs=3) as expool,
        tc.tile_pool(name="aps", bufs=2, space="PSUM") as aps,
        tc.tile_pool(name="small", bufs=4) as small,
    ):
        for h in range(HEADS):
            qk_t = h // 4
            hp = h % 4
            # expand bias strips -> full per-head B [128(j), jc, i]: the
            # partition group a = xj%4 of chunk jc holds strip window
            # starting at block 31-xj (xj = 4*jc + a).
            # 4 custom-AP DMAs: src strip[yj, (31-4jc-a)*32+c] over (yj,jc,c)
            # (overlapping jc windows, stride -128) -> dst B[32a+yj, jc, c]
            B_sb = bpool.tile([128, 8 * N_TOK], F16, tag="B")
            VP = bass_rust.VecI64Pair
            for a in range(4):
                src = wsv[h].copy()
                src.ap = VP([(2016, 32), (-128, 8), (1, N_TOK)])
                src.offset = wsv[h].offset + 992 - 32 * a
                dst = B_sb[:].copy()
                dst.ap = VP([(8 * N_TOK, 32), (N_TOK, 8), (1, N_TOK)])
                dst.offset = B_sb[:].offset + 32 * a * 8 * N_TOK
                # SWDGE (Pool queue): keeps these off the in-order SP DMA
                # queue, which stalls behind the stats-collective chain
                nc.gpsimd.dma_start(dst, src)
            sT = []
            for jc in range(8):
                st = stpool.tile([128, 2 * N_TOK], F16, tag="sT")
                kpos = 32 * hp
                tp = (96, 0) if hp == 3 else None
                for img in range(IMGS):
                    dots = aps.tile([128, N_TOK], F32, tag="dots")
                    for ih in range(2):
                        nc.tensor.matmul(
                            dots[:, ts(ih, 512)],
                            lhsT=QKb[2 + qk_t][kpos:kpos + 32,
                                               img * N_TOK + jc * 128:
                                               img * N_TOK + jc * 128 + 128],
                            rhs=QKb[qk_t][kpos:kpos + 32,
                                          img * N_TOK + ih * 512:
                                          img * N_TOK + ih * 512 + 512],
                            start=True, stop=True,
                            tile_position=tp,
                        )
                    ex = expool.tile([128, N_TOK], F16, tag="exp")
                    nc.scalar.activation(out=ex[:], in_=dots[:], func=AF.Exp)
                    nc.vector.tensor_mul(
                        st[:, ts(img, N_TOK)], ex[:], B_sb[:, ts(jc, N_TOK)],
                    )
                sT.append(st)
            half = 64 * (h % 2)
            d = h // 2
            for img in range(IMGS):
                # attn @ V_aug: rows 0..63 = dv, row 64 = rowsum (ones col)
                outp = aps.tile([128, N_TOK], F32, tag="outT", name="outp")
                for ih in range(2):
                    for jc in range(8):
                        nc.tensor.matmul(
                            outp[0:65, ts(ih, 512)],
                            lhsT=V_aug[:, img * 8 + jc, h, 1:66],
                            rhs=sT[jc][:, img * N_TOK + ih * 512:
                                       img * N_TOK + ih * 512 + 512],
                            start=(jc == 0), stop=(jc == 7),
                        )
                rowi = small.tile([1, N_TOK], F32, tag="rowi")
                nc.vector.reciprocal(rowi[:], outp[64:65, :])
                bc = small.tile([64, N_TOK], F32, tag="bc")
                nc.gpsimd.partition_broadcast(bc[:], rowi[0:1, :])
                nc.vector.tensor_mul(
                    gXp[d][half:half + 64, ts(img, N_TOK)], outp[0:64, :], bc[:]
                )

        # deferred gelu over head-pair tiles: one ActFuncSet switch total,
        # per-partition scale/bias = folded V BatchNorm affine
        for d in range(4):
            nc.scalar.activation(
                out=gTp[d][:], in_=gXp[d][:], func=AF.Gelu_apprx_tanh,
                bias=bvp[:, d:d + 1], scale=svp[:, d:d + 1],
            )

    # ---------------- phase D: out-projection + BN2 ------------------------
    with (
        tc.tile_pool(name="zps", bufs=2, space="PSUM") as zps,
        tc.tile_pool(name="sps", bufs=1, space="PSUM") as sps,
        tc.tile_pool(name="zmisc", bufs=2) as zmisc,
        tc.tile_pool(name="dram2", bufs=1, space="DRAM") as dram2,
        tc.tile_pool(name="fin", bufs=1) as fin,
    ):
        sums_ps = sps.tile([1, 2 * DIM], F32)
        for t in range(16):
            ps = zps.tile([128, DIM], F32, tag="z")
            for dc in range(4):
                nc.tensor.matmul(
                    ps[:],
                    lhsT=gTp[dc][:, ts(t, 128)],
                    rhs=wop[dc][:],
                    start=(dc == 0), stop=(dc == 3),
                )
            nc.vector.tensor_copy(out=z_sb[:, ts(t, DIM)], in_=ps[:])
            z2 = zmisc.tile([128, DIM], F32, tag="z2")
            nc.gpsimd.tensor_mul(z2[:], z_sb[:, ts(t, DIM)], z_sb[:, ts(t, DIM)])
            nc.tensor.matmul(
                sums_ps[0:1, 0:DIM], lhsT=onescol[:], rhs=z_sb[:, ts(t, DIM)],
                start=(t == 0), stop=(t == 15), skip_group_check=True,
            )
            nc.tensor.matmul(
                sums_ps[0:1, DIM:2 * DIM], lhsT=onescol[:], rhs=z2[:],
                start=(t == 0), stop=(t == 15), skip_group_check=True,
            )
        st2 = fin.tile([1, 2 * DIM], F32)
        nc.vector.tensor_copy(out=st2[:], in_=sums_ps[:])
        cin = dram2.tile([1, 2 * DIM], F32)
        cout = dram2.tile([1, 2 * DIM], F32)
        nc.sync.dma_start(cin[:], st2[:])
        if os.environ.get("KTIME"):
            nc.sync.dma_start(cout[:], cin[:])
        else:
            nc.gpsimd.collective_compute(
                "AllReduce", ALU.add, replica_groups=RG,
                ins=[cin[:].opt()], outs=[cout[:].opt()],
            )
        st2a = fin.tile([1, 2 * DIM], F32)
        nc.sync.dma_start(st2a[:], cout[:])

        # finalize BN2 on [1, 256] rows.  z_true = z_raw + b_out
        mean = fin.tile([1, DIM], F32)
        ex2 = fin.tile([1, DIM], F32)
        veps = fin.tile([1, DIM], F32)
        sq0 = fin.tile([1, DIM], F32)
        tmp = fin.tile([1, DIM], F32)
        s2 = fin.tile([1, DIM], F32)
        b2f = fin.tile([1, DIM], F32)
        b_out_row = vec2_sb[0:1, 0:DIM]
        go_row = vec2_sb[0:1, DIM:2 * DIM]
        bo_row = vec2_sb[0:1, 2 * DIM:3 * DIM]
        nc.vector.tensor_scalar_mul(mean[:], st2a[0:1, 0:DIM], 1.0 / NTOT)
        nc.vector.tensor_scalar_mul(ex2[:], st2a[0:1, DIM:2 * DIM], 1.0 / NTOT)
        # ex2_true = ex2 + 2*mean*b_out + b_out^2 ; m_true = mean + b_out
        nc.vector.scalar_tensor_tensor(
            out=tmp[:], in0=mean[:], scalar=2.0, in1=b_out_row,
            op0=ALU.mult, op1=ALU.mult,
        )
        nc.vector.tensor_add(ex2[:], ex2[:], tmp[:])
        nc.vector.tensor_mul(tmp[:], b_out_row, b_out_row)
        nc.vector.tensor_add(ex2[:], ex2[:], tmp[:])
        m_true = fin.tile([1, DIM], F32)
        nc.vector.tensor_add(m_true[:], mean[:], b_out_row)
        nc.vector.scalar_tensor_tensor(
            out=tmp[:], in0=m_true[:], scalar=-1.0, in1=m_true[:],
            op0=ALU.mult, op1=ALU.mult,
        )
        nc.vector.tensor_add(veps[:], ex2[:], tmp[:])
        nc.vector.tensor_scalar_add(veps[:], veps[:], EPS)
        nc.scalar.sqrt(sq0[:], veps[:])
        nc.vector.reciprocal(tmp[:], sq0[:])        # rstd2
        nc.vector.tensor_mul(s2[:], go_row, tmp[:])
        # bias2_final = bo - mean_raw * s2
        nc.vector.scalar_tensor_tensor(
            out=tmp[:], in0=mean[:], scalar=-1.0, in1=s2[:],
            op0=ALU.mult, op1=ALU.mult,
        )
        nc.vector.tensor_add(b2f[:], bo_row, tmp[:])
        # fold int8 output quantization into the BN2 affine
        nc.vector.tensor_scalar_mul(s2[:], s2[:], OUT_Q)
        nc.vector.tensor_scalar_mul(b2f[:], b2f[:], OUT_Q)
        # broadcast the [1,256] affine rows across partitions once; the
        # per-chunk ops re-read them via stride-0 t-dim APs
        bcs2 = fin.tile([128, DIM], F32)
        bcb2 = fin.tile([128, DIM], F32)
        nc.gpsimd.partition_broadcast(bcs2[:], s2[0:1, :])
        nc.gpsimd.partition_broadcast(bcb2[:], b2f[0:1, :])
        # apply affine + int8 quantization in 4 chunks so the output DMAs
        # overlap the remaining DVE work instead of trailing one big op
        zt = fin.tile([128, 16 * DIM], F32)
        zo = fin.tile([128, 16 * DIM], I8)
        odv = out_d.rearrange("(t p) c -> p t c", p=128)
        ztv = zt.rearrange("p (t c) -> p t c", t=16)
        zsv = z_sb.rearrange("p (t c) -> p t c", t=16)
        zov = zo.rearrange("p (t c) -> p t c", t=16)
        scb = bcs2[:].unsqueeze(1).broadcast_to((128, 4, DIM))
        bib = bcb2[:].unsqueeze(1).broadcast_to((128, 4, DIM))
        for q in range(4):
            nc.vector.tensor_mul(
                ztv[:, 4 * q:4 * q + 4, :], zsv[:, 4 * q:4 * q + 4, :], scb
            )
            nc.vector.tensor_add(
                zov[:, 4 * q:4 * q + 4, :], ztv[:, 4 * q:4 * q + 4, :], bib
            )
            nc.sync.dma_start(odv[:, 4 * q:4 * q + 4, :], zov[:, 4 * q:4 * q + 4, :])


def kernel(**inputs):
    f = np.float32
    x = np.asarray(inputs["x"], f)
    wq, wk, wv = (np.asarray(inputs[k], f) for k in ("wq", "wk", "wv"))
    pos_emb = np.asarray(inputs["pos_emb"], f)
    w_out = np.asarray(inputs["w_out"], f)

    # Toeplitz strips of exp(bias/scale): [yj, t=31+s, yi, h] with
    # value exp(pos_emb[|s|*32 + |yj-yi|, h] / SCALE); core c carries head c.
    E = np.exp(pos_emb / SCALE)                      # [1024, h]
    dy = np.abs(np.arange(32)[:, None] - np.arange(32)[None, :])  # [yj, yi]
    dxs = np.abs(np.arange(63) - 31)                 # [t]
    idx = dxs[None, :, None] * 32 + dy[:, None, :]   # [yj, t, yi]
    strips = E[idx].astype(np.float16)               # [yj, t, yi, h]

    wqkv = np.concatenate([wq, wk, wv], axis=1).astype(np.float16)  # [256, 1024]
    wout16 = w_out.astype(np.float16)
    # gb: col 0-7 gammas, 8-15 betas, chunk order q0 q1 k0 k1 v0..v3
    gcat = np.concatenate(
        [np.asarray(inputs["gq"], f), np.asarray(inputs["gk"], f),
         np.asarray(inputs["gv"], f)]
    ).reshape(8, 128).T
    bcat = np.concatenate(
        [np.asarray(inputs["bq"], f), np.asarray(inputs["bk"], f),
         np.asarray(inputs["bv"], f)]
    ).reshape(8, 128).T
    gb = np.concatenate([gcat, bcat], axis=1).astype(np.float16)  # [128, 16]
    vec2 = np.concatenate(
        [np.asarray(inputs["b_out"], f), np.asarray(inputs["go"], f),
         np.asarray(inputs["bo"], f)]
    ).astype(np.float16)                              # [768]

    if "nc" not in _cache:
        _enable_jax_compile_cache()
        _cache["nc"] = _build()
    nc = _cache["nc"]

    xs = x.reshape(16, N_TOK, DIM).astype(np.float16)
    in_maps = []
    for c in range(NCORES):
        pk = np.empty((1, PK_N), np.float16)
        pk[0, OFF_QKV:OFF_QKV + 32768] = wqkv[32 * c:32 * c + 32].ravel()
        pk[0, OFF_STRIP:OFF_STRIP + 64512] = strips[:, :, :, c].ravel()
        pk[0, OFF_WOUT:OFF_WOUT + 16384] = wout16[64 * c:64 * c + 64].ravel()
        pk[0, OFF_GB:OFF_GB + 2048] = gb.ravel()
        pk[0, OFF_VEC2:OFF_VEC2 + 768] = vec2
        in_maps.append({
            "x": np.ascontiguousarray(xs[2 * c:2 * c + 2].reshape(TOKS, DIM)),
            "wpk": pk,
        })

    res = run_bass_kernel_spmd(
        nc, in_maps, core_ids=list(range(NCORES)),
        trace=bool(int(os.environ.get("KTRACE", "0"))),
    )
    _cache["res"] = res
    out = np.concatenate([r["out"] for r in res.results], axis=0)
    return (out.astype(np.float32) / OUT_Q).reshape(16, FMAP, FMAP, DIM)


if __name__ == "__main__":
    if os.environ.get("BUILD_ONLY"):
        _build()
        print("BUILD OK")

